# revision 60
# baseline (speedup 1.0000x reference)
"""Trainium2 Bass kernel for the two-tower GCN (nn_GCN2).

Distribution: nodes partitioned by destination range across 8 cores
(graph parallel). Edge lists are preprocessed on host (index manipulation
only): assigned to the core owning their dst node, dst-sorted, and padded
per dst TILE to a cross-core-uniform slot count so every core runs the
identical program with identical selector windows. Gather indices are
stored relative to the table midpoint (signed int16 sign-extends in the
gather ucode), so one index stream covers all 50176 table rows.

All floating-point math runs on device across 3 SPMD launches:
  A: xW   = x @ [W1|W3]  (f16 matmul, fp8 output table)
  B: h^T  = relu(spmm(A, xW) + b), o = hW2      (fp8 gather, 4096/instr)
  C: oT   = spmm(A, hW2); gated fusion; log_softmax

The irregular segment-sum runs as selector-matrix matmuls on the tensor
engine. Key cost levers vs the naive formulation:
  - gathers batch 4096 indices per instruction (single_packet=False),
    amortizing the per-instruction SWDGE overhead on the Pool engine;
  - the launch-B gather table is fp8 (halves gather DMA bytes);
  - launch-C gathers only the 40 used columns per row (elem_size=40);
  - selectors are narrow: 128 dst-sorted edges span only ~9-22 distinct
    dst columns (avg degree 16), so sel is [128, 32] not [128, 128]. The
    per-tile PSUM accumulator is bias-initialized by a rank-1 matmul with
    start=True, and each chunk accumulates into a 32-wide column window
    (launch B: free-dim slice; launch C: 32-aligned partition slice).
"""
from contextlib import ExitStack

import numpy as np

import concourse.bass as bass
import concourse.tile as tile
from concourse import bacc, mybir
from concourse.bass_utils import run_bass_kernel_spmd
from concourse.masks import make_identity

P = 128
NCORES = 8
N = 50000
NFEAT = 512
NHID = 128
NCLASS = 40
NLOC = N // NCORES            # 6250 real nodes per core
NTILE = (NLOC + P - 1) // P   # 49 dst tiles per core
NLOCP = NTILE * P             # 6272 padded rows per core
NPAD = NCORES * NLOCP         # 50176 padded table rows
MID = NPAD // 2               # gather base row (signed idx spans the table)
G = 32                        # chunks per gather batch
R = G * P                     # 4096 indices per dma_gather (ucode max with
                              # single_packet=False; 8192 returns bad data)

f8 = mybir.dt.float8e4
f16 = mybir.dt.float16
f32 = mybir.dt.float32
i16 = mybir.dt.int16
i32 = mybir.dt.int32
ACT = mybir.ActivationFunctionType
ALU = mybir.AluOpType

NP_F8 = mybir.dt.np(f8)
C_ROT = 40             # first chunk index eligible for ACT/Pool sel builds
C_DIVIDE = False       # DVE divide fails walrus codegen; keep reciprocal
XW_F8 = True                  # launch-B gather table in fp8 (else f16)


def _cdiv(a, b):
    return (a + b - 1) // b


# ---------------------------------------------------------------- host prep

class TowerPlan:
    """Edge preprocessing for one tower (one graph).

    Edges are dst-sorted per core and padded per dst tile to the max
    per-tile count over cores, so slot ranks map to the same dst quantiles
    on every core and a single compiled program (with baked-in selector
    windows) serves all 8 cores. 128-slot chunks then span only a narrow
    dst range (measured max 22 across cores for this problem size).

    Per chunk c the plan stores:
      winB[c]  : unaligned window base (min dst over cores)
      wB[c]    : B selector width (32, or 64 for outliers)
      winC[c]  : 32-aligned window base
      wC[c]    : C selector width (multiple of 32)
      evB[c]   : [(tile, sel_lo, sel_hi, psum_lo, first, last)]
      evC[c]   : [(tile, sel_lo, sel_hi, part_off, first, last)]
    idx        : [NCORES, nb, 128, R//16] int16 mid-relative gather indices
    dlB/dlC/vl : [NCORES, 128, nchunks] f16 window-relative dstloc / vals
    """

    def __init__(self, edge_index, edge_vals):
        src = np.asarray(edge_index[0]).astype(np.int64)
        dst = np.asarray(edge_index[1]).astype(np.int64)
        vals = np.asarray(edge_vals).astype(np.float32)

        core = dst // NLOC
        ldst = dst - core * NLOC

        # per-(core, tile) sorted dst lists; uniform per-tile slot count
        percore = []
        tilecnt = np.zeros((NCORES, NTILE), np.int64)
        for c in range(NCORES):
            ld = np.sort(ldst[core == c])
            percore.append(ld)
            tilecnt[c] = np.bincount(ld // P, minlength=NTILE)
        K = tilecnt.max(axis=0)                       # [NTILE]
        starts = np.r_[0, np.cumsum(K)]
        nslot = int(starts[-1])
        self.nchunks = _cdiv(nslot, P)
        nslotp = self.nchunks * P
        self.nb = _cdiv(self.nchunks, G)

        # slot arrays per core: srcrel / ldst / val (pad: idx 0, val 0)
        srcrel = np.zeros((NCORES, nslotp), np.int32)
        lda = np.full((NCORES, nslotp), -1, np.int64)   # -1 = pad
        vla = np.zeros((NCORES, nslotp), np.float32)
        for c in range(NCORES):
            m = core == c
            o = np.lexsort((np.arange(m.sum()), ldst[m]))
            s_src, s_ld, s_vl = src[m][o], ldst[m][o], vals[m][o]
            tptr = np.r_[0, np.cumsum(np.bincount(s_ld // P,
                                                  minlength=NTILE))]
            for t in range(NTILE):
                a, b = tptr[t], tptr[t + 1]
                sl = starts[t]
                srcrel[c, sl:sl + (b - a)] = s_src[a:b] - MID
                lda[c, sl:sl + (b - a)] = s_ld[a:b]
                vla[c, sl:sl + (b - a)] = s_vl[a:b]

        ldc = lda.reshape(NCORES, self.nchunks, P)
        real = ldc >= 0
        cmin = np.where(real, ldc, 10**9).min(axis=(0, 2))   # [nchunks]
        cmax = np.where(real, ldc, -1).max(axis=(0, 2))
        dead = cmax < 0                      # all-pad chunk (tail only)
        cmin[dead] = 0
        cmax[dead] = 0
        self.live = ~dead
        span = cmax - cmin + 1
        assert span.max() <= 64, f"chunk dst span {span.max()} > 64"

        self.winB = cmin.copy()
        self.wB = np.where(span <= 32, 32, 64)
        self.winC = (cmin // 32) * 32
        self.wC = ((cmax - self.winC) // 32 + 1) * 32

        # per-chunk matmul events; is_first / is_last annotated afterwards so
        # exactly ONE event per tile carries each flag (a chunk can emit two
        # events into the same tile on the C side)
        self.evB = [[] for _ in range(self.nchunks)]
        self.evC = [[] for _ in range(self.nchunks)]
        for c in range(self.nchunks):
            if not self.live[c]:
                continue
            w, t0, t1 = self.winB[c], cmin[c] // P, cmax[c] // P
            for t in range(t0, t1 + 1):
                lo = max(w, t * P)
                hi = min(w + self.wB[c], (t + 1) * P, NLOCP)
                self.evB[c].append([t, lo - w, hi - w, lo - t * P,
                                    False, False])
            a = self.winC[c]
            for blk in range(a, a + self.wC[c], 32):
                if blk >= NLOCP or blk + 32 <= cmin[c] or blk > cmax[c]:
                    continue
                t = blk // P
                self.evC[c].append([t, blk - a, blk - a + 32, blk - t * P,
                                    False, False])
        for evs in (self.evB, self.evC):
            first_seen, last_seen = {}, {}
            for c in range(self.nchunks):
                for i, e in enumerate(evs[c]):
                    if e[0] not in first_seen:
                        first_seen[e[0]] = (c, i)
                    last_seen[e[0]] = (c, i)
            assert len(first_seen) == NTILE
            for t, (c, i) in first_seen.items():
                evs[c][i][4] = True
            for t, (c, i) in last_seen.items():
                evs[c][i][5] = True

        # dl (window-relative, pads at 0 with val 0) and vl, f16 column-major
        dlB = np.where(real, ldc - self.winB[None, :, None], 0)
        dlC = np.where(real, ldc - self.winC[None, :, None], 0)

        # live chunk count per batch (trailing dead chunks need no gather)
        nb = self.nb
        self.bchunks = []
        nlive = int(np.flatnonzero(self.live)[-1]) + 1
        for b in range(nb):
            self.bchunks.append(max(0, min(nlive - b * G, G)))
        assert self.bchunks[-1] > 0

        # wrapped idx [NCORES, nb, 128, R//16]; the gather ucode stops at the
        # last non-negative index, so ensure each batch's final GATHERED slot
        # is >= 0 by swapping within its chunk (slot order in a chunk is free)
        for cc in range(NCORES):
            for b in range(nb):
                last = b * R + self.bchunks[b] * P - 1
                if srcrel[cc, last] >= 0:
                    continue
                c0 = (last // P) * P
                j = c0 + int(np.argmax(srcrel[cc, c0:last + 1] >= 0))
                assert srcrel[cc, j] >= 0, "all-negative chunk tail"
                for arr in (srcrel, vla):
                    arr[cc, [j, last]] = arr[cc, [last, j]]
                for arr in (dlB, dlC):
                    arr.reshape(NCORES, -1)[cc, [j, last]] = \
                        arr.reshape(NCORES, -1)[cc, [last, j]]

        w = np.zeros((NCORES, nb * R), np.int32)
        w[:, :nslotp] = srcrel
        w = w.reshape(NCORES, nb, R)
        jj = np.arange(R)
        wr = np.zeros((NCORES, nb, 16, R // 16), np.int16)
        wr[:, :, jj % 16, jj // 16] = w.astype(np.int16)
        self.idx = np.ascontiguousarray(np.tile(wr, (1, 1, 8, 1)))

        def colmaj(a):
            out = a.reshape(NCORES, self.nchunks, P).astype(np.float32)
            return np.ascontiguousarray(out.transpose(0, 2, 1))

        self.dlB = colmaj(dlB)
        self.dlC = colmaj(dlC)
        self.vl = colmaj(vla)


# ---------------------------------------------------------------- kernels

def _dma_gather_small(gp, out_ap, in_ap, idxs_ap, num_idxs, num_idxs_reg,
                      elem_size, elem_step, queue_num=0):
    """dma_gather for elem sizes below 256B (non-transpose DRAM path only).

    bass.dma_gather asserts elem_size_bytes % 256 == 0, but that alignment is
    only required by the transpose ucode; the plain path only needs the row
    stride in 256B units. Mirrors bass.py's lowering minus that assert.
    single_packet=False enables batches up to 4096 indices.
    """
    from concourse import ap_utils
    from concourse._compat import exact_div
    assert idxs_ap.dtype == i16
    assert in_ap.dtype == out_ap.dtype
    assert in_ap.space == bass.MemorySpace.DRAM
    assert ap_utils.ap_is_contiguous(out_ap.ap[1:])
    assert ap_utils.ap_is_contiguous(idxs_ap.ap[1:])
    assert in_ap.ap[-1][1] == out_ap.ap[-1][1] == elem_size
    assert out_ap.ap[0][1] * out_ap.ap[1][1] == _cdiv(num_idxs, P) * P
    assert in_ap.ap[0][0] == elem_step
    stride_bytes = elem_step * mybir.dt.size(in_ap.dtype)
    stride_bytes_256 = exact_div(stride_bytes, 256)
    _in_ap = gp.lower_ap_dma(in_ap, for_custom_bir_dma=True)
    _idxs_ap = gp.lower_ap(idxs_ap)
    _out_ap = gp.lower_ap(out_ap)
    return gp.add_instruction(mybir.InstDMAGatherAnt(
        name=gp.bass.get_next_instruction_name(),
        ins=[*_in_ap, _idxs_ap, gp.lower_val_access(gp.to_reg(num_idxs_reg))],
        outs=[_out_ap],
        transpose=False, num_idxs=num_idxs, elem_size=elem_size,
        stride_bytes_256=stride_bytes_256, gen_mode=0, single_packet=False,
        queue_num=queue_num, sbuf_tokens_per_rank=0, sbuf_free_dim_per_rank=0,
        sbuf_free_dim_pad_per_rank=0, sbuf_byte_offset=0,
    ))


def _iota_const(nc, ctx, tc):
    pool = ctx.enter_context(tc.tile_pool(name="iotac", bufs=1))
    it32 = pool.tile([P, P], i32)
    nc.gpsimd.iota(it32[:], pattern=[[1, P]], base=0, channel_multiplier=0)
    it16 = pool.tile([P, P], f16)
    nc.vector.tensor_copy(it16[:], it32[:])
    return it16


def build_A(nc):
    xT = nc.dram_tensor("xT", [NFEAT, NLOCP], f32, kind="ExternalInput").ap()
    w13 = nc.dram_tensor("w13", [NFEAT, 2 * NHID], f32, kind="ExternalInput").ap()
    odt = f8 if XW_F8 else f16
    # partition-major output: [p, t, f]; host permutes rows back. Keeps each
    # DMA descriptor's contiguous run at bs*256B (>= 512B, no 2x penalty).
    out = nc.dram_tensor("out", [P, NTILE, 2 * NHID], odt,
                         kind="ExternalOutput").ap()
    KCH = NFEAT // P  # 4

    TB = 7                    # max dst tiles per column block
    # small first block for a fast ramp; taper the tail so the post-DMA
    # compute drain is short
    SIZES = [2, 5, 7, 7, 7, 7, 7, 4, 3]
    NBLK = len(SIZES)

    # One DMA per k-pair per block loads 2 k-chunks (HWDGE fixed cost is
    # per DMA); f32 -> f16 conversion per half, split across DVE and ACT.
    # (float32r matmuls would skip the conversions but fail walrus codegen.)
    xTr = xT.rearrange("(k p) t -> p k t", k=KCH)
    with tile.TileContext(nc) as tc, ExitStack() as ctx:
        big = ctx.enter_context(tc.tile_pool(name="big", bufs=1))
        xf_pool = ctx.enter_context(tc.tile_pool(name="xf", bufs=4))
        psum = ctx.enter_context(tc.tile_pool(name="ps", bufs=4, space="PSUM"))

        xt0 = xf_pool.tile([P, KCH, TB * P], f32, tag="xt", name="xt0")
        for h in (0, 1):
            nc.sync.dma_start(xt0[:, 2 * h:2 * h + 2, 0:SIZES[0] * P],
                              xTr[:, 2 * h:2 * h + 2, 0:SIZES[0] * P])
        w_f = big.tile([P, KCH, 2 * NHID], f32, tag="wf")
        nc.sync.dma_start(w_f[:], w13.rearrange("(k p) f -> p k f", k=KCH))
        w_t = big.tile([P, KCH, 2 * NHID], f16, tag="w")
        nc.vector.tensor_copy(w_t[:], w_f[:])
        ob = big.tile([P, NTILE, 2 * NHID], odt, tag="ob")

        t0 = 0
        for blk in range(NBLK):
            bs = SIZES[blk]
            if blk == 0:
                xt = xt0
            else:
                xt = xf_pool.tile([P, KCH, TB * P], f32, tag="xt")
                for h in (0, 1):
                    nc.sync.dma_start(
                        xt[:, 2 * h:2 * h + 2, 0:bs * P],
                        xTr[:, 2 * h:2 * h + 2, t0 * P:(t0 + bs) * P]
                    )
            xb = xf_pool.tile([P, KCH, TB * P], f16, tag="xb")
            nc.vector.tensor_copy(xb[:, 0:2, 0:bs * P], xt[:, 0:2, 0:bs * P])
            nc.scalar.copy(xb[:, 2:4, 0:bs * P], xt[:, 2:4, 0:bs * P])
            for rr in range(bs):
                r = t0 + rr
                ps = psum.tile([P, 2 * NHID], f32, tag="ps")
                for k in range(KCH):
                    nc.tensor.matmul(
                        ps[:],
                        lhsT=xb[:, k, rr * P:(rr + 1) * P],
                        rhs=w_t[:, k, :],
                        start=(k == 0), stop=(k == KCH - 1),
                    )
                # alternate eviction engine so neither serializes the drain
                if r % 2 == 0:
                    nc.vector.tensor_copy(ob[:, r, :], ps[:])
                else:
                    nc.scalar.copy(ob[:, r, :], ps[:])
            nc.sync.dma_start(out[:, t0:t0 + bs, :], ob[:, t0:t0 + bs, :])
            t0 += bs
    nc.compile()
    return nc


EDGE_HB = 10                   # head: gather batches loaded up front


def _load_edge_inputs(nc, ctx, tc, plans, bulk=False, post_load=None):
    """Declare + load idx/dl/vl heads for both towers; return a per-batch
    tail loader. Tail slices are emitted from the batch loop so their
    DMA_ENGINES requests queue behind the gathers they should not starve.
    With bulk=True the whole tail loads up front instead (right for a
    launch whose DMA queue is the bottleneck anyway).
    """
    idx_t, dl_t, vl_t = {}, {}, {}
    tail_emit = {}
    pool = ctx.enter_context(tc.tile_pool(name="edges", bufs=1))
    for tw in (0, 1):
        plan = plans[tw]
        nb, nch = plan.nb, plan.nchunks
        d_idx = nc.dram_tensor(
            f"idx{tw}", [nb, P, R // 16], i16, kind="ExternalInput"
        ).ap()
        t_idx = pool.tile([P, nb, R // 16], i16, tag=f"idx{tw}")
        d_idx_r = d_idx.rearrange("b p w -> p b w")
        hb = min(EDGE_HB, nb)
        nc.sync.dma_start(t_idx[:, 0:hb, :], d_idx_r[:, 0:hb, :])
        idx_t[tw] = t_idx
        d_dl = nc.dram_tensor(
            f"dl{tw}", [P, nch], f32, kind="ExternalInput"
        ).ap()
        t_dl = pool.tile([P, nch], f32, tag=f"dl{tw}")
        nc.sync.dma_start(t_dl[:, 0:hb * G], d_dl[:, 0:hb * G])
        dl_t[tw] = t_dl
        d_vl = nc.dram_tensor(
            f"vl{tw}", [P, nch], f32, kind="ExternalInput"
        ).ap()
        t_vl = pool.tile([P, nch], f32, tag=f"vl{tw}")
        nc.sync.dma_start(t_vl[:, 0:hb * G], d_vl[:, 0:hb * G])
        vl_t[tw] = t_vl
        if post_load is not None:
            post_load(tw, 0, min(hb * G, nch), t_dl, t_vl)

        if bulk:
            if nb > hb:
                nc.sync.dma_start(t_idx[:, hb:nb, :], d_idx_r[:, hb:nb, :])
                nc.sync.dma_start(t_dl[:, hb * G:nch], d_dl[:, hb * G:nch])
                nc.sync.dma_start(t_vl[:, hb * G:nch], d_vl[:, hb * G:nch])
            tail_emit[tw] = lambda b: None
            continue

        def emit(b, tw=tw, hb=hb, nb=nb, nch=nch, d_idx_r=d_idx_r,
                 d_dl=d_dl, d_vl=d_vl, t_idx=t_idx, t_dl=t_dl, t_vl=t_vl):
            # load batch b+hb's inputs while batch b gathers
            tb = b + hb
            if tb >= nb:
                return
            c0, c1 = tb * G, min((tb + 1) * G, nch)
            nc.sync.dma_start(t_idx[:, tb:tb + 1, :], d_idx_r[:, tb:tb + 1, :])
            nc.sync.dma_start(t_dl[:, c0:c1], d_dl[:, c0:c1])
            nc.sync.dma_start(t_vl[:, c0:c1], d_vl[:, c0:c1])
            if post_load is not None:
                post_load(tw, c0, c1, t_dl, t_vl)
        tail_emit[tw] = emit
    return idx_t, dl_t, vl_t, tail_emit


def _emit_batch(nc, state, tw, b):
    """Emit gather + selector builds for batch b of tower tw."""
    plan, pools = state["plans"][tw], state["pools"]
    iota = state["iota"]
    nbq = state["q"]
    state["q"] += 1

    elem = state["elem"]
    gcnt = plan.bchunks[b]
    nidx = gcnt * P
    msgs = pools["msgs"].tile([P, G, elem], state["mdt"], tag="msgs")
    _dma_gather_small(
        nc.gpsimd, msgs[:, 0:gcnt, :], state["tabs"][tw],
        state["idx"][tw][:, b, 0:nidx // 16],
        num_idxs=nidx, num_idxs_reg=nidx,
        elem_size=elem, elem_step=state["tab_step"],
        queue_num=nbq % 2,
    )
    sel = pools["sel"].tile([P, G, state["selw"]], f16, tag="sel")
    dl = state["dl"][tw]
    vl = state["vl"][tw]
    wsel = state["wsel"]
    rot = state.get("rot")
    for g in range(gcnt):
        c = b * G + g
        if not plan.live[c]:
            continue
        wd = wsel(plan, c)
        eng = "dve"
        if rot is not None:
            cnt = state["cnt"]
            state["cnt"] += 1
            if cnt >= rot:
                r = cnt % 12
                eng = "pool" if r == 3 else ("act" if r == 7 else "dve")
        if eng == "act":
            # onehot via |iota - dl| on the scalar engine: sel =
            # Relu(vl - vl*|iota - dl|) is exactly vl at iota==dl, else 0
            ndl = state["ndl"][tw]
            nvl = state["nvl"][tw]
            a = pools["abs"].tile([P, 96], f16, tag="abs")
            nc.scalar.activation(out=a[:, 0:wd], in_=iota[:, 0:wd],
                                 func=ACT.Abs, bias=ndl[:, c:c + 1],
                                 scale=1.0)
            nc.scalar.activation(out=sel[:, g, 0:wd], in_=a[:, 0:wd],
                                 func=ACT.Relu, bias=vl[:, c:c + 1],
                                 scale=nvl[:, c:c + 1])
        elif eng == "pool":
            nc.gpsimd.tensor_scalar(
                out=sel[:, g, 0:wd], in0=iota[:, 0:wd],
                scalar1=dl[:, c:c + 1], scalar2=vl[:, c:c + 1],
                op0=ALU.is_equal, op1=ALU.mult,
            )
        else:
            nc.vector.tensor_scalar(
                out=sel[:, g, 0:wd], in0=iota[:, 0:wd],
                scalar1=dl[:, c:c + 1], scalar2=vl[:, c:c + 1],
                op0=ALU.is_equal, op1=ALU.mult,
            )
    return msgs, sel


def build_B(nc, plans):
    tdt = f8 if XW_F8 else f16
    xw = nc.dram_tensor("xw", [NPAD, 2 * NHID], tdt, kind="ExternalInput").ap()
    w24 = nc.dram_tensor("w24", [NHID, 2 * NCLASS], f16, kind="ExternalInput").ap()
    b13 = nc.dram_tensor("b13", [1, 2 * NHID], f16, kind="ExternalInput").ap()
    out = nc.dram_tensor("out", [P, NTILE, 2 * NCLASS], f16,
                         kind="ExternalOutput").ap()

    with tile.TileContext(nc) as tc, ExitStack() as ctx:
        iota = _iota_const(nc, ctx, tc)
        idx_t, dl_t, vl_t, tail_emit = _load_edge_inputs(nc, ctx, tc,
                                                         plans, bulk=True)
        consts = ctx.enter_context(tc.tile_pool(name="consts", bufs=1))
        w24_t = consts.tile([NHID, 2 * NCLASS], f16)
        nc.sync.dma_start(w24_t[:], w24[:])
        b13_t = consts.tile([1, 2 * NHID], f16)
        nc.sync.dma_start(b13_t[:], b13[:])
        ones_t = consts.tile([1, P], f16, tag="ones")
        nc.vector.memset(ones_t[:], 1.0)
        ob = consts.tile([P, NTILE, 2 * NCLASS], f16, tag="ob")

        pools = {
            "msgs": ctx.enter_context(tc.tile_pool(name="msgs", bufs=4)),
            "sel": ctx.enter_context(tc.tile_pool(name="sel", bufs=4)),
        }
        psum = ctx.enter_context(tc.tile_pool(name="ps", bufs=5, space="PSUM"))
        psum_o = ctx.enter_context(tc.tile_pool(name="pso", bufs=2, space="PSUM"))
        hpool = ctx.enter_context(tc.tile_pool(name="h", bufs=3))

        state = {
            "plans": plans, "pools": pools, "iota": iota, "q": 0,
            "idx": idx_t, "dl": dl_t, "vl": vl_t,
            "tabs": [xw[MID:, 0:NHID], xw[MID:, NHID:2 * NHID]],
            "tab_step": 2 * NHID,
            "elem": NHID,
            "mdt": tdt,
            "selw": 64,
            "wsel": lambda plan, c: plan.wB[c],
        }

        done = [set(), set()]
        ps_tiles = {}
        for b in range(max(plans[0].nb, plans[1].nb)):
          ms = {}
          for tw in (0, 1):
            if b < plans[tw].nb:
                ms[tw] = _emit_batch(nc, state, tw, b)
                tail_emit[tw](b)
          for tw in (0, 1):
            plan = plans[tw]
            if b >= plan.nb:
                continue
            msgs, sel = ms[tw]
            for g in range(plan.bchunks[b]):
                c = b * G + g
                for t, slo, shi, plo, is_f, is_l in plan.evB[c]:
                    if is_f:
                        ps_h = psum.tile([NHID, P], f32, tag="psh",
                                         name=f"psh{tw}_{t}")
                        ps_tiles[(tw, t)] = ps_h
                        # rank-1 bias init: ps[hid, dst] = b1[hid] (start)
                        nc.tensor.matmul(
                            ps_h[:], lhsT=b13_t[:, tw * NHID:(tw + 1) * NHID],
                            rhs=ones_t[:], start=True, stop=False,
                        )
                    nc.tensor.matmul(
                        ps_tiles[(tw, t)][:, plo:plo + (shi - slo)],
                        lhsT=msgs[:, g, :],
                        rhs=sel[:, g, slo:shi],
                        start=False, stop=is_l,
                    )
                    if not is_l:
                        continue
                    ps_h = ps_tiles.pop((tw, t))
                    hT = hpool.tile([NHID, P], f16, tag="hT")
                    nc.scalar.activation(
                        out=hT[:], in_=ps_h[:], func=ACT.Relu, scale=1.0,
                    )
                    ps_o = psum_o.tile([P, NCLASS], f32, tag="pso")
                    nc.tensor.matmul(
                        ps_o[:], lhsT=hT[:],
                        rhs=w24_t[:, tw * NCLASS:(tw + 1) * NCLASS],
                        start=True, stop=True,
                    )
                    nc.scalar.copy(ob[:, t, tw * NCLASS:(tw + 1) * NCLASS],
                                   ps_o[:])
                    done[tw].add(t)
                    # stream output per tile group once both towers finished
                    # every tile of the group; per-tile for the tail so the
                    # drain doesn't wait on a whole 7-tile group
                    if t in done[1 - tw]:
                        if t >= 42:
                            nc.sync.dma_start(out[:, t:t + 1, :],
                                              ob[:, t:t + 1, :])
                        else:
                            lo = t - t % 7
                            hi = min(lo + 7, 42)
                            grp = set(range(lo, hi))
                            if grp <= (done[0] & done[1]):
                                nc.sync.dma_start(out[:, lo:hi, :],
                                                  ob[:, lo:hi, :])
    nc.compile()
    return nc


def build_C(nc, plans):
    # hw2 table: f16, 256B row stride; tower tw's 40 cols start at tw*64
    hw2 = nc.dram_tensor("hw2", [NPAD, 128], f16, kind="ExternalInput").ap()
    # wl padded to 128 rows: rows 0:40 = Wl[0:40], rows 64:104 = Wl[40:80]
    wl = nc.dram_tensor("wl", [P, NCLASS], f16, kind="ExternalInput").ap()
    b24 = nc.dram_tensor("b24", [1, 2 * NCLASS], f16, kind="ExternalInput").ap()
    nbl = nc.dram_tensor("nbl", [NCLASS, 1], f32, kind="ExternalInput").ap()
    out = nc.dram_tensor("out", [P, NTILE, NCLASS], f32,
                         kind="ExternalOutput").ap()

    with tile.TileContext(nc) as tc, ExitStack() as ctx:
        iota = _iota_const(nc, ctx, tc)
        negp = ctx.enter_context(tc.tile_pool(name="neg", bufs=1))
        ndl_t, nvl_t = {}, {}
        for tw in (0, 1):
            nch = plans[tw].nchunks
            ndl_t[tw] = negp.tile([P, nch], f32, tag=f"ndl{tw}",
                                  name=f"ndl{tw}")
            nvl_t[tw] = negp.tile([P, nch], f32, tag=f"nvl{tw}",
                                  name=f"nvl{tw}")

        def _negate_slice(tw, c0, c1, t_dl, t_vl):
            # ACT-built selectors need -dl and -vl; negate each dl/vl slice
            # as its load lands (slices load during the batch stream)
            nc.vector.tensor_scalar(
                out=ndl_t[tw][:, c0:c1], in0=t_dl[:, c0:c1],
                scalar1=-1.0, scalar2=None, op0=ALU.mult)
            nc.vector.tensor_scalar(
                out=nvl_t[tw][:, c0:c1], in0=t_vl[:, c0:c1],
                scalar1=-1.0, scalar2=None, op0=ALU.mult)

        idx_t, dl_t, vl_t, tail_emit = _load_edge_inputs(
            nc, ctx, tc, plans, post_load=_negate_slice)
        consts = ctx.enter_context(tc.tile_pool(name="consts", bufs=1))
        wl_t = consts.tile([P, NCLASS], f16)
        nc.sync.dma_start(wl_t[:], wl[:])
        b24_t = consts.tile([1, 2 * NCLASS], f16)
        nc.sync.dma_start(b24_t[:], b24[:])
        nbl_t = consts.tile([NCLASS, 1], f32)
        nc.sync.dma_start(nbl_t[:], nbl[:])
        ones_t = consts.tile([1, P], f16, tag="ones")
        nc.vector.memset(ones_t[:], 1.0)
        ident = consts.tile([P, P], f16, tag="ident")
        make_identity(nc, ident[:])
        # fused per-tower outputs [dst, (tw0 cols 0:40 | tw1 cols 64:104)];
        # zeroed once so the cat transpose emits clean zero fill rows
        o_cat = consts.tile([P, NTILE, P], f16, tag="o_cat")
        nc.vector.memset(o_cat[:], 0.0)
        # logits + softmax stats, ln'd once at the end
        l_all = consts.tile([P, NTILE, NCLASS], f16, tag="l_all")
        ob = consts.tile([P, NTILE, NCLASS], f32, tag="ob")
        negmax_all = consts.tile([P, NTILE], f32, tag="negmax")
        esum_all = consts.tile([P, NTILE], f32, tag="esum")
        lse_all = consts.tile([P, NTILE], f32, tag="lse")

        pools = {
            "msgs": ctx.enter_context(tc.tile_pool(name="msgs", bufs=4)),
            "sel": ctx.enter_context(tc.tile_pool(name="sel", bufs=4)),
            "abs": ctx.enter_context(tc.tile_pool(name="absw", bufs=4)),
        }
        work = ctx.enter_context(tc.tile_pool(name="work", bufs=6))

        ROT = C_ROT            # head chunks stay on DVE (negations pending)

        state = {
            "plans": plans, "pools": pools, "iota": iota, "q": 0,
            "idx": idx_t, "dl": dl_t, "vl": vl_t,
            "ndl": ndl_t, "nvl": nvl_t, "rot": ROT, "cnt": 0,
            "tabs": [hw2[MID:, 0:NCLASS], hw2[MID:, 64:64 + NCLASS]],
            "tab_step": 128,
            "elem": NCLASS,
            "mdt": f16,
            "selw": 96,
            "wsel": lambda plan, c: plan.wC[c],
        }

        acc_pool = ctx.enter_context(tc.tile_pool(name="acc", bufs=4,
                                                  space="PSUM"))
        eps = ctx.enter_context(tc.tile_pool(name="eps", bufs=2, space="PSUM"))

        def fuse_tile(t):
            # gated fusion + log_softmax stats for tile t. Only Exp runs on
            # the scalar engine here -- any other activation function would
            # trigger a 1.3us table reload per switch.
            o1 = o_cat[:, t, 0:NCLASS]
            o2 = o_cat[:, t, 64:64 + NCLASS]
            dif = work.tile([P, NCLASS], f16, tag="dif")
            nc.vector.tensor_tensor(out=dif[:], in0=o1, in1=o2,
                                    op=ALU.subtract)
            # catT [128, P]: one transpose; fill rows come out zero
            ps_cat = eps.tile([P, P], f16, tag="cat", bufs=2)
            nc.tensor.transpose(out=ps_cat[:], in_=o_cat[:, t, :],
                                identity=ident[:])
            catT = work.tile([P, P], f16, tag="catT")
            nc.scalar.copy(catT[:], ps_cat[:])
            # gate^T = sigmoid(z + bl) = 1 / (1 + exp(-(z + bl)))  [C, P]
            ps_z = eps.tile([NCLASS, P], f32, tag="z", bufs=1)
            nc.tensor.matmul(ps_z[:], lhsT=wl_t[:], rhs=catT[:],
                             start=True, stop=True)
            eneg = work.tile([NCLASS, P], f16, tag="eneg")
            nc.scalar.activation(out=eneg[:], in_=ps_z[:], func=ACT.Exp,
                                 bias=nbl_t[:], scale=-1.0)
            gt = work.tile([NCLASS, P], f16, tag="gt")
            nc.vector.tensor_scalar(out=gt[:], in0=eneg[:], scalar1=1.0,
                                    scalar2=None, op0=ALU.add)
            if not C_DIVIDE:
                with nc.allow_low_precision(reason="gate in (0,1); f16 ok"):
                    nc.vector.reciprocal(gt[:], gt[:])
            # denom^T (or gate^T) [P, C] via PE transpose
            ps_g = eps.tile([P, NCLASS], f16, tag="g", bufs=1)
            nc.tensor.transpose(out=ps_g[:], in_=gt[:],
                                identity=ident[0:NCLASS, 0:NCLASS])
            # out = o2 + (o1 - o2) / (1 + exp(-(z + bl)))
            with nc.allow_low_precision(reason="gate in (0,1); f16 ample"):
                nc.vector.tensor_tensor(
                    out=dif[:], in0=dif[:], in1=ps_g[:],
                    op=ALU.divide if C_DIVIDE else ALU.mult)
            nc.vector.tensor_tensor(out=l_all[:, t, :], in0=o2, in1=dif[:],
                                    op=ALU.add)
            nc.vector.tensor_reduce(
                out=negmax_all[:, t:t + 1], in_=l_all[:, t, :],
                axis=mybir.AxisListType.X, op=ALU.max, negate=True,
            )
            etmp = work.tile([P, NCLASS], f16, tag="etmp")
            nc.scalar.activation(
                out=etmp[:], in_=l_all[:, t, :], func=ACT.Exp,
                bias=negmax_all[:, t:t + 1], scale=1.0,
                accum_out=esum_all[:, t:t + 1],
            )

        done = [set(), set()]
        tleft = [NTILE]                 # fused-tile countdown
        flushed = [0]                   # first unflushed tile
        FLUSH_AT = (24, 40, 46, NTILE)  # tiles-fused thresholds

        def flush(hi):
            lo = flushed[0]
            if hi <= lo:
                return
            nc.scalar.activation(out=lse_all[:, lo:hi],
                                 in_=esum_all[:, lo:hi], func=ACT.Ln)
            for u in range(lo, hi):
                nc.vector.tensor_scalar(
                    out=ob[:, u, :], in0=l_all[:, u, :],
                    scalar1=negmax_all[:, u:u + 1],
                    scalar2=lse_all[:, u:u + 1],
                    op0=ALU.add, op1=ALU.subtract,
                )
            nc.sync.dma_start(out[:, lo:hi, :], ob[:, lo:hi, :])
            flushed[0] = hi

        def finish_tile(t):
            fuse_tile(t)
            tleft[0] -= 1
            nfused = NTILE - tleft[0]
            if nfused in FLUSH_AT:
                # tiles complete roughly in order; flush the finished prefix
                both = done[0] & done[1]
                hi = flushed[0]
                while hi < NTILE and hi in both:
                    hi += 1
                flush(hi)

        ps_tiles = {}
        for b in range(max(plans[0].nb, plans[1].nb)):
            ms = {}
            for tw in (0, 1):
                if b < plans[tw].nb:
                    ms[tw] = _emit_batch(nc, state, tw, b)
                    tail_emit[tw](b)
            for tw in (0, 1):
                plan = plans[tw]
                if b >= plan.nb:
                    continue
                msgs, sel = ms[tw]
                for g in range(plan.bchunks[b]):
                    c = b * G + g
                    for t, slo, shi, poff, is_f, is_l in plan.evC[c]:
                        if is_f:
                            # NOTE: start=True resets PSUM at bank granularity
                            # on hardware, so concurrent accumulators must not
                            # share a bank -- one pool buf per (tower, tile).
                            ps_o = acc_pool.tile([P, NCLASS], f32, tag="acc",
                                                 name=f"acc{tw}_{t}")
                            ps_tiles[(tw, t)] = ps_o
                            # rank-1 bias init: ps[dst, c] = b2[c] (start)
                            nc.tensor.matmul(
                                ps_o[:], lhsT=ones_t[:],
                                rhs=b24_t[:, tw * NCLASS:(tw + 1) * NCLASS],
                                start=True, stop=False,
                            )
                        nc.tensor.matmul(
                            ps_tiles[(tw, t)][poff:poff + 32, :],
                            lhsT=sel[:, g, slo:shi],
                            rhs=msgs[:, g, 0:NCLASS],
                            start=False, stop=is_l,
                            tile_position=(0, poff),
                        )
                        if not is_l:
                            continue
                        ps_o = ps_tiles.pop((tw, t))
                        nc.scalar.copy(
                            o_cat[:, t, tw * 64:tw * 64 + NCLASS], ps_o[:])
                        done[tw].add(t)
                        if t in done[1 - tw]:
                            finish_tile(t)
    nc.compile()
    return nc


# ---------------------------------------------------------------- driver

TRACE = False          # set by test.py to collect per-launch artifacts
LAST_NCS = []          # built Bass modules per launch when TRACE


def _run(nc, in_maps):
    if TRACE and nc not in LAST_NCS:
        LAST_NCS.append(nc)
    # the emulated runtime very occasionally returns corrupted (non-finite)
    # buffers; the program is deterministic, so retry the launch if so
    for attempt in range(3):
        res = run_bass_kernel_spmd(nc, in_maps, core_ids=list(range(NCORES)))
        ok = all(np.isfinite(arr.astype(np.float32)).all()
                 for r in res.results for arr in r.values())
        if ok:
            return res
    return res


def _make_nc():
    return bacc.Bacc(
        "TRN2", target_bir_lowering=False, debug=False,
        num_devices=NCORES, num_swdge_queues=2,
    )


def kernel(x, edge_index, edge_vals, edge_index2, edge_vals2,
           W1, b1, W2, b2, W3, b3, W4, b4, Wl, bl):
    x = np.asarray(x, np.float32)
    plans = [TowerPlan(edge_index, edge_vals), TowerPlan(edge_index2, edge_vals2)]

    def edge_inmap(c, dl_key):
        m = {}
        for tw in (0, 1):
            m[f"idx{tw}"] = plans[tw].idx[c]
            m[f"dl{tw}"] = getattr(plans[tw], dl_key)[c]
            m[f"vl{tw}"] = plans[tw].vl[c]
        return m

    # ---- launch A: xW = x @ [W1|W3]
    w13 = np.concatenate([np.asarray(W1, np.float32),
                          np.asarray(W3, np.float32)], axis=1)
    nc = _make_nc()
    build_A(nc)
    in_maps = []
    for c in range(NCORES):
        xT = np.zeros((NFEAT, NLOCP), np.float32)
        xT[:, :NLOC] = x[c * NLOC:(c + 1) * NLOC].T
        in_maps.append({"xT": xT, "w13": w13})
    res = _run(nc, in_maps)
    xw = np.zeros((NPAD, 2 * NHID), NP_F8 if XW_F8 else np.float16)
    for c in range(NCORES):
        arr = res.results[c]["out"].transpose(1, 0, 2).reshape(NLOCP, 2 * NHID)
        xw[c * NLOC:(c + 1) * NLOC] = arr[:NLOC]

    # ---- launch B: h = relu(spmm(xW) + b); hW2
    w24 = np.concatenate([np.asarray(W2, np.float32),
                          np.asarray(W4, np.float32)], axis=1).astype(np.float16)
    b13 = np.concatenate([np.asarray(b1, np.float32),
                          np.asarray(b3, np.float32)]).reshape(1, 2 * NHID)
    b13 = b13.astype(np.float16)
    nc = _make_nc()
    build_B(nc, plans)
    in_maps = [{"xw": xw, "w24": w24, "b13": b13, **edge_inmap(c, "dlB")}
               for c in range(NCORES)]
    res = _run(nc, in_maps)
    hw2 = np.zeros((NPAD, 128), np.float16)
    for c in range(NCORES):
        o = res.results[c]["out"].transpose(1, 0, 2).reshape(
            NLOCP, 2 * NCLASS)[:NLOC]
        hw2[c * NLOC:(c + 1) * NLOC, 0:NCLASS] = o[:, 0:NCLASS]
        hw2[c * NLOC:(c + 1) * NLOC, 64:64 + NCLASS] = o[:, NCLASS:2 * NCLASS]

    # ---- launch C: o = spmm(hW2) + b; gated fusion; log_softmax
    wl_f = np.asarray(Wl, np.float32).astype(np.float16)      # [2C, C]
    wl_h = np.zeros((P, NCLASS), np.float16)
    wl_h[0:NCLASS] = wl_f[0:NCLASS]
    wl_h[64:64 + NCLASS] = wl_f[NCLASS:2 * NCLASS]
    b24 = np.stack([np.asarray(b2, np.float32),
                    np.asarray(b4, np.float32)]).reshape(1, 2 * NCLASS)
    b24 = b24.astype(np.float16)
    nbl_c = -np.asarray(bl, np.float32).reshape(NCLASS, 1)
    nc = _make_nc()
    build_C(nc, plans)
    in_maps = [{"hw2": hw2, "wl": wl_h, "b24": b24, "nbl": nbl_c,
                **edge_inmap(c, "dlC")}
               for c in range(NCORES)]
    res = _run(nc, in_maps)
    out = np.zeros((N, NCLASS), np.float32)
    for c in range(NCORES):
        arr = res.results[c]["out"].transpose(1, 0, 2).reshape(NLOCP, NCLASS)
        out[c * NLOC:(c + 1) * NLOC] = arr[:NLOC]
    return out
